# revision 11
# baseline (speedup 1.0000x reference)
"""Trainium2 Bass kernel for the differentiable-Kalman-filter loss.

Math: the reference runs a T=100000-step linear recurrence
  x_{i+1} = M x_i + K obs[i-1],  eps_i = obs[i] - C x_{i+1},  M = A - K C
and accumulates yvar = sum outer(eps_i) + decaying P-terms, loss = slogdet(yvar/T).
rho(M) ~ 0.963, so the recurrence has ~400-step memory: eps becomes a truncated
causal convolution of obs.  Each core computes eps for a 12160-row slab via a
two-level blocked conv (B=16 within-block taps as one 512x512 triangular matmul,
block-boundary states from J=24 block-level taps), then accumulates the Gram
E^T E on-chip.  The first W=2720 rows + the tiny P-series are computed exactly
on host in f64 (they need the exact initial transient and cost ~nothing).
"""
import numpy as np

T, N, B, J, W, NCORES = 100000, 32, 16, 24, 2720, 8
R = (T - W) // NCORES       # rows per core = 12160
NB = R // B                 # 760 blocks per core
PSI = NB + J                # 784 panel columns (incl halo)
NTS = 6                     # s-tiles per core
PS = [128, 128, 128, 128, 128, 120]
J0S = [0, 4, 8, 12, 16, 20]

_PROG_CACHE = {}


def _build_device_consts(A64, C64, K64):
    import ml_dtypes
    bf16 = ml_dtypes.bfloat16
    M = A64 - K64 @ C64
    Mp = [np.eye(N)]
    for _ in range(B + 1):
        Mp.append(M @ Mp[-1])
    H = [C64 @ Mp[k] @ K64 for k in range(B)]
    TrilHneg = np.zeros((512, 512))
    for r in range(B):
        for t in range(r, B):
            TrilHneg[r*N:(r+1)*N, t*N:(t+1)*N] = -H[t - r].T
    Gmat = np.zeros((512, N))
    for r in range(B):
        Gmat[r*N:(r+1)*N, :] = (Mp[B-1-r] @ K64).T
    CMn = np.zeros((N, 512))
    for t in range(B):
        CMn[:, t*N:(t+1)*N] = -(C64 @ Mp[t+1]).T
    MB = Mp[B]
    D = [np.eye(N)]
    for _ in range(J - 1):
        D.append(MB @ D[-1])
    trilh = np.ascontiguousarray(TrilHneg.reshape(4, 128, 512).transpose(1, 0, 2).reshape(128, 2048)).astype(bf16)
    gmat = np.ascontiguousarray(Gmat.reshape(4, 128, N).transpose(1, 0, 2).reshape(128, 128)).astype(bf16)
    cmn = np.ascontiguousarray(CMn).astype(bf16)
    dstk = np.zeros((128, 192))
    for jg in range(6):
        for rho in range(4):
            dstk[32*rho:32*rho+32, 32*jg:32*jg+32] = D[4*jg + rho].T
    dstk = dstk.astype(bf16)
    identb = np.eye(32).astype(bf16)
    return trilh, gmat, cmn, dstk, identb


def _host_exact(obs, A64, C64, K64, x0, Psqrt0):
    """f64 exact: P-series + outer(obs0) + eps outers for i < W."""
    obs64 = obs.astype(np.float64)
    M = A64 - K64 @ C64
    Y = np.outer(obs64[0], obs64[0])
    P = Psqrt0.astype(np.float64)
    for _ in range(4000):
        CP = C64 @ P
        Y += CP @ CP.T
        P = M @ P
        if np.abs(P).max() < 1e-16:
            break
    x = x0.astype(np.float64)
    for i in range(W):
        o_prev = obs64[i - 1] if i > 0 else obs64[T - 1]
        x = M @ x + K64 @ o_prev
        eps = obs64[i] - C64 @ x
        Y += np.outer(eps, eps)
    return Y


def _patch_tile_drain():
    """This walrus build allows only one sem wait per Drain; split the
    TileContext tail drain's waits across multiple drain instructions."""
    import concourse.tile as tile
    from concourse.vector_clock import ScopedClock
    if getattr(tile.TileContext, "_kf_drain_patched", False):
        return
    def _drain_and_barrier(self, tick_clock, wait_clock):
        nc = self.nc
        drain_inst = nc.sync.drain()
        wait_clock.add_sem_waits(drain_inst.ins, ScopedClock({None: tick_clock.global_clock}))
        si = drain_inst.ins.sync_info
        waits = list(si.on_wait or [])
        if len(waits) > 1:
            si.on_wait = waits[:1]
            for i in range(1, len(waits)):
                extra = nc.sync.drain()
                esi = extra.ins.sync_info
                if esi is None:
                    extra.ins.sync_info = type(si)(on_wait=waits[i:i+1], on_update=[])
                else:
                    esi.on_wait = waits[i:i+1]
        nc.all_engine_barrier(sem_only=True)
        assert self.sems is not None
        popped = nc._tile_sem_poison_stack.pop()
        assert popped is self._sem_poison
        nc.clear_and_free_semaphores(list(self.sems.allocated().values()))
    tile.TileContext._drain_and_barrier = _drain_and_barrier
    tile.TileContext._kf_drain_patched = True


def _split_multi_waits(nc):
    """This walrus build encodes at most one sem wait per instruction; hoist
    extra waits onto NoOps inserted just before in the same engine stream."""
    import concourse.mybir as mybir
    for func in nc.m.functions:
        for blk in func.blocks:
            insts = blk.instructions
            out, changed = [], False
            for inst in insts:
                si = inst.sync_info
                waits = list(si.on_wait) if si and si.on_wait else []
                if len(waits) > 1:
                    changed = True
                    for k, w in enumerate(waits[:-1]):
                        out.append(mybir.InstNoOp(
                            name=f"{inst.name}-hw{k}", engine=inst.engine,
                            bass_nofuse=True,
                            sync_info=mybir.SyncInfo(on_wait=[w], on_update=[])))
                    si.on_wait = [waits[-1]]
                out.append(inst)
            if changed:
                blk.instructions = out


def build_program(debug=False):
    import concourse.bass as bass
    import concourse.mybir as mybir
    import concourse.tile as tile
    _patch_tile_drain()
    f32 = mybir.dt.float32
    bf16 = mybir.dt.bfloat16

    nc = bass.Bass()
    # host-prepped bf16 inputs: obsb = Oblk tiles, pans = transposed panels (+halo)
    obsb_in = nc.declare_dram_parameter("obsb", [128, 6 * 512], bf16, isOutput=False)
    pans_in = nc.declare_dram_parameter("pans", [128, 4 * 768 + 4 * 32], bf16, isOutput=False)
    trilh_in = nc.declare_dram_parameter("trilh", [128, 2048], bf16, isOutput=False)
    gmat_in = nc.declare_dram_parameter("gmat", [128, 128], bf16, isOutput=False)
    cmn_in = nc.declare_dram_parameter("cmn", [32, 512], bf16, isOutput=False)
    dstk_in = nc.declare_dram_parameter("dstk", [128, 192], bf16, isOutput=False)
    identb_in = nc.declare_dram_parameter("identb", [32, 32], bf16, isOutput=False)
    yout = nc.declare_dram_parameter("yout", [128, 512], f32, isOutput=True)
    if debug:
        dbg_gt = nc.declare_dram_parameter("dbg_gt", [32, PSI], f32, isOutput=True)
        dbg_xbt = nc.declare_dram_parameter("dbg_xbt", [32, NB], f32, isOutput=True)
        dbg_e0 = nc.declare_dram_parameter("dbg_e0", [128, 512], f32, isOutput=True)

    HAL = 4 * 768   # halo column offset inside pans

    with tile.TileContext(nc) as tc:
        with (
            tc.tile_pool(name="consts", bufs=1) as cpool,
            tc.tile_pool(name="obs", bufs=1) as opool,
            tc.tile_pool(name="work", bufs=1) as wpool,
            tc.tile_pool(name="etile", bufs=3) as epool,
            tc.tile_pool(name="trps", bufs=2, space="PSUM") as trpool,
            tc.tile_pool(name="epsum", bufs=3, space="PSUM") as eppool,
            tc.tile_pool(name="gramps", bufs=1, space="PSUM") as gpool,
        ):
            # ---- inputs: two big DMAs on separate HWDGE rings, consts via SWDGE
            pans = opool.tile([128, 4 * 768 + 4 * 32], bf16)
            nc.sync.dma_start(pans[:, HAL : HAL+128], pans_in[:, HAL : HAL+128])
            for kc in range(4):
                nc.sync.dma_start(pans[:, kc*768 : (kc+1)*768],
                                  pans_in[:, kc*768 : (kc+1)*768])
            obsb = opool.tile([128, 6 * 512], bf16)
            nc.scalar.dma_start(obsb[:], obsb_in[:])
            gmat = cpool.tile([128, 128], bf16)
            nc.gpsimd.dma_start(gmat[:], gmat_in[:])
            cmn = cpool.tile([32, 512], bf16)
            nc.gpsimd.dma_start(cmn[:], cmn_in[:])
            dstk = cpool.tile([128, 192], bf16)
            nc.gpsimd.dma_start(dstk[:], dstk_in[:])
            identb = cpool.tile([32, 32], bf16)
            nc.gpsimd.dma_start(identb[:], identb_in[:])
            trilh = cpool.tile([128, 2048], bf16)
            nc.gpsimd.dma_start(trilh[:, 0:1024], trilh_in[:, 0:1024])
            nc.gpsimd.dma_start(trilh[:, 1024:2048], trilh_in[:, 1024:2048])

            # ---- gT [32, 784]: halo part + main part
            gth_ps = trpool.tile([32, 32], f32, tag="trps")
            for kc in range(4):
                nc.tensor.matmul(gth_ps[:, 0:24],
                                 lhsT=gmat[:, 32*kc : 32*kc+32],
                                 rhs=pans[:, HAL + 32*kc : HAL + 32*kc+24],
                                 start=(kc == 0), stop=(kc == 3))
            gtm_ps = trpool.tile([32, NB], f32, tag="trps")
            for c0, nn_ in [(0, 512), (512, NB - 512)]:
                for kc in range(4):
                    nc.tensor.matmul(gtm_ps[:, c0 : c0+nn_],
                                     lhsT=gmat[:, 32*kc : 32*kc+32],
                                     rhs=pans[:, kc*768 + c0 : kc*768 + c0 + nn_],
                                     start=(kc == 0), stop=(kc == 3))
            gts = wpool.tile([32, PSI], bf16)
            nc.vector.tensor_copy(gts[:, 0:24], gth_ps[:, 0:24])
            nc.vector.tensor_copy(gts[:, 24:PSI], gtm_ps[:])

            # ---- gS [128, 784]: group rho = gT shifted right by rho cols
            gs_ps = trpool.tile([128, PSI], f32, tag="trps")
            for rho in range(4):
                tp = (0, 32 * rho) if rho else None
                for c0, c1 in [(rho, 512), (512, PSI)]:
                    nc.tensor.matmul(gs_ps[32*rho : 32*rho+32, c0:c1],
                                     lhsT=identb[:],
                                     rhs=gts[:, c0-rho : c1-rho],
                                     start=True, stop=True, tile_position=tp)
            gss = wpool.tile([128, PSI], bf16)
            nc.vector.tensor_copy(gss[:], gs_ps[:])

            # ---- XbT [32, 760]: sum_j D_j g_{s+23-j} via 6 tap-groups of 4
            xbt_ps = trpool.tile([32, NB], f32, tag="trps")
            for jg, j0 in enumerate(J0S):
                for c0, nn_ in [(0, 512), (512, NB - 512)]:
                    nc.tensor.matmul(xbt_ps[:, c0 : c0+nn_],
                                     lhsT=dstk[:, 32*jg : 32*jg+32],
                                     rhs=gss[:, (23-j0)+c0 : (23-j0)+c0+nn_],
                                     start=(j0 == 0), stop=(j0 == J0S[-1]))
            xbt = wpool.tile([32, NB], bf16)
            nc.vector.tensor_copy(xbt[:], xbt_ps[:])

            # ---- conv + E + Gram
            gram_ps = gpool.tile([128, 512], f32)
            for st in range(NTS):
                p = PS[st]
                eps_ps = eppool.tile([128, 512], f32, tag="epsum")
                for kc in range(4):
                    nc.tensor.matmul(eps_ps[:p, :],
                                     lhsT=pans[:, kc*768 + 128*st : kc*768 + 128*st + p],
                                     rhs=trilh[:, 512*kc : 512*kc+512],
                                     start=(kc == 0), stop=False)
                nc.tensor.matmul(eps_ps[:p, :],
                                 lhsT=xbt[:, 128*st : 128*st+p],
                                 rhs=cmn[:, :],
                                 start=False, stop=True)
                esb = epool.tile([128, 512], bf16, tag="etile")
                nc.vector.tensor_add(esb[:p, :],
                                     obsb[:p, 512*st : 512*st+512],
                                     eps_ps[:p, :])
                if debug and st == 0:
                    nc.sync.dma_start(dbg_e0[:], esb[:])
                for g in range(4):
                    # start=True zeroes the full 2KB bank row per written
                    # partition, so only the very first matmul may set it.
                    nc.tensor.matmul(gram_ps[:, 128*g : 128*g+128],
                                     lhsT=esb[:p, 128*g : 128*g+128],
                                     rhs=esb[:p, 128*g : 128*g+128],
                                     start=(st == 0 and g == 0),
                                     stop=(st == NTS - 1 and g == 3),
                                     skip_group_check=True)

            ysb = wpool.tile([128, 512], f32)
            nc.vector.tensor_copy(ysb[:], gram_ps[:])
            nc.sync.dma_start(yout[:], ysb[:])
            if debug:
                nc.sync.dma_start(dbg_gt[:], gts[:])
                nc.sync.dma_start(dbg_xbt[:], xbt[:])

    _split_multi_waits(nc)
    return nc


def _core_inputs(obs, c, consts):
    """Host-side layout prep for one core: bf16 Oblk tiles + transposed panels."""
    import ml_dtypes
    bf16 = ml_dtypes.bfloat16
    trilh, gmat, cmn, dstk, identb = consts
    start = W + c * R
    flat = obs[start - 385 : start + R]                 # [12545, 32]
    # Oblk tiles: rows [start+16s, +16) for s in [0, 760)
    ob = np.zeros((768, 512), np.float32)
    ob[:NB] = flat[385 : 385 + R].reshape(NB, 512)
    obsb = np.ascontiguousarray(
        ob.reshape(6, 128, 512).transpose(1, 0, 2).reshape(128, 6 * 512)).astype(bf16)
    # panel rows (shifted by -1 obs row): s in [0, 760)
    pm = np.zeros((768, 512), np.float32)
    pm[:NB] = flat[384 : 384 + R].reshape(NB, 512)
    ptm = pm.reshape(768, 4, 128).transpose(2, 1, 0)    # [128, 4, 768]
    pth = np.zeros((128, 4, 32), np.float32)
    ph = flat[0:384].reshape(24, 512)                   # halo panel rows
    pth[:, :, :24] = ph.reshape(24, 4, 128).transpose(2, 1, 0)
    pans = np.ascontiguousarray(np.concatenate(
        [ptm.reshape(128, 4 * 768), pth.reshape(128, 4 * 32)], axis=1)).astype(bf16)
    return {"obsb": obsb, "pans": pans, "trilh": trilh, "gmat": gmat,
            "cmn": cmn, "dstk": dstk, "identb": identb}


def kernel(observations, A, C, K, x0, Psqrt0, _trace=False, _trace_kwargs=None):
    obs = np.ascontiguousarray(observations, np.float32)
    A64 = np.asarray(A, np.float64)
    C64 = np.asarray(C, np.float64)
    K64 = np.asarray(K, np.float64)

    consts = _build_device_consts(A64, C64, K64)
    Y = _host_exact(obs, A64, C64, K64, np.asarray(x0), np.asarray(Psqrt0))

    if "prog" not in _PROG_CACHE:
        _PROG_CACHE["prog"] = build_program()
    nc = _PROG_CACHE["prog"]

    in_maps = [_core_inputs(obs, c, consts) for c in range(NCORES)]

    from concourse.bass_utils import run_bass_kernel_spmd
    kw = dict(_trace_kwargs or {})
    res = run_bass_kernel_spmd(nc, in_maps, list(range(NCORES)), trace=_trace, **kw)

    for c in range(NCORES):
        G = np.asarray(res.results[c]["yout"], np.float64)
        for g in range(4):
            for tau in range(4):
                Y += G[32*tau:32*tau+32, 128*g+32*tau : 128*g+32*tau+32]
    loss = np.linalg.slogdet(Y / T)[1]
    out = np.float32(loss)
    if _trace:
        return out, res
    return out
